# revision 12
# baseline (speedup 1.0000x reference)
"""DisagreementRegularizer Trainium2 kernel (v4).

reference math:
    xn = x / max(||x||_2 along d, eps)
    out[b] = -mean_{q,p} (xn @ xn^T)  =  -(1/Q^2) * || sum_q xn[b,q,:] ||^2

Measured op costs on TRN2 drove this design ([128,256] f16 tile units):
    ACT Square big-call   ~245 ns/tile   (only square engine besides DVE)
    DVE x*x (TT 2x mode)  ~180 ns/tile
    DVE fold-tree + TR    ~220 ns/tile   (cheapest row-sum; TR alone ~290,
                                          tensor_scalar cache-reduce ~420,
                                          ACT Square+accum_out ~736,
                                          tensor_tensor_reduce hangs HW)
Per core (16 batches): 64 tiles of square+row-sum, split so ACT squares
~56 tiles and DVE squares 8 plus all fold-trees, both ~15us busy.

Pipeline (7 groups):
  * host casts x to fp16 (identical numerics to the SWDGE cast path);
    4.2MB/core streamed via HWDGE on the sync ring, issued upfront.
  * ACT Square(g) and DVE TT-square(g) write one shared sq buffer;
    the DVE fold chain for group g runs ONE GROUP LATE so it never
    waits on the ACT square mid-stream.
  * rnorm = sqrt(1/sumsq): DVE reciprocal + ACT Sqrt (fp16). On ACT,
    sqrt(g) is emitted after Square(g+1) to keep the square stream hot.
  * s[b]: PE matmuls, x chunk stationary + rnorm as 1-column moving ->
    [128,1] PSUM columns spread across partitions; per-group PSUM bank.
    Warmup/filler matmuls hold the PE p-state between bursts.
  * per-group [128,2GB] PSUM->SBUF copies (on ACT) gather into one
    [128,32] tile; 3 consolidated stores ship it.

Host: out[b] = -(1/Q^2) * sum_d s[b,d]^2.
Sharding: pure data parallel, batch dim 128 -> 16 per core x 8 cores.
"""

import numpy as np

B, Q, D = 128, 512, 256
N_CORES = 8
BL = B // N_CORES  # 16 batches per core
CHUNKS = 4  # 512 rows = 128 partitions x 4 row-chunks
EPS = 1e-12

# (batches, n_DVE_square_tiles) per group; ACT squares the rest.
GROUPS = [(2, 2), (3, 1), (3, 1), (3, 1), (2, 1), (2, 1), (1, 1)]
assert sum(g for g, _ in GROUPS) == BL
STORE_AFTER = (2, 5, 6)  # consolidated stores after these groups' copies

N_WARM = 7   # [1,512] warmup matmuls during the first load
N_FILL = 10  # [1,128] filler matmuls per group gap


def _build(nc):
    import concourse.mybir as mybir
    import concourse.tile as tile

    f32 = mybir.dt.float32
    f16 = mybir.dt.float16
    AF = mybir.ActivationFunctionType
    ALU = mybir.AluOpType

    x_d = nc.dram_tensor("x", [BL, Q, D], f16, kind="ExternalInput").ap()
    # s_out[p, 2*b+h] = s[b, 128*h + p]
    s_d = nc.dram_tensor("s_out", [128, 2 * BL], f32, kind="ExternalOutput").ap()

    NG = len(GROUPS)
    with tile.TileContext(nc) as tc:
        with (
            tc.tile_pool(name="xp", bufs=1) as xp,
            tc.tile_pool(name="small", bufs=1) as small,
            tc.tile_pool(name="ps", bufs=1, space="PSUM") as psp,
        ):
            # ---- t0 ------------------------------------------------------
            warm_w = small.tile([128, 512], f16, tag="warm_w")
            nc.vector.memset(warm_w[:], 0.125)
            dummy = small.tile([1, 1], f32, tag="dummy")
            nc.vector.memset(dummy[:], 1.0)
            dummy2 = small.tile([1, 1], f32, tag="dummy2")
            nc.scalar.activation(out=dummy2[:], in_=dummy[:], func=AF.Sqrt)

            x_tiles = []
            b0 = 0
            for g, (GB, _) in enumerate(GROUPS):
                x_t = xp.tile([128, GB, CHUNKS, D], f16, tag=f"x_t{g}")
                # gpsimd (SWDGE) ring: the sync/SP queue is blocked for the
                # first ~7us by the NEFF preamble's constant TENSOR_LOADs
                src = x_d[b0 : b0 + GB].rearrange("b (p c) d -> p b c d", p=128)
                nc.gpsimd.dma_start(out=x_t[:], in_=src)
                x_tiles.append(x_t)
                b0 += GB

            warm_ps = psp.tile([128, 512], f32, tag="warm_ps")
            s_ps = [
                psp.tile([128, 512], f32, tag=f"s_ps{g}", name=f"s_ps{g}")
                for g in range(NG)
            ]
            for _ in range(N_WARM):
                nc.tensor.matmul(
                    warm_ps[0:1, 0:512], warm_w[:, 0:1], warm_w[:, 0:512],
                    start=True, stop=True,
                )

            s_all = small.tile([128, 2 * BL], f32, tag="s_all")

            # ---- software-pipelined groups -------------------------------
            # stage A (group g): ACT+DVE squares into sq(g)
            # stage B (group g, one group later): DVE folds -> sumsq ->
            #   recip; ACT sqrt; PE burst; ACT copy.
            sq_tiles = {}
            meta = {}
            b0 = 0
            for g, (GB, ND) in enumerate(GROUPS):
                NT = 4 * GB
                meta[g] = (GB, ND, NT, b0)
                b0 += GB

            def squares(g):
                GB, ND, NT, _ = meta[g]
                xf = x_tiles[g][:].rearrange("p b c d -> p (b c) d")
                sq = small.tile(
                    [128, NT, D], f16, tag="sq", bufs=2, name=f"sq{g}"
                )
                nc.vector.tensor_tensor(
                    out=sq[:, 0:ND, :], in0=xf[:, 0:ND, :], in1=xf[:, 0:ND, :],
                    op=ALU.mult,
                )
                nc.scalar.activation(
                    out=sq[:, ND:NT, :], in_=xf[:, ND:NT, :], func=AF.Square,
                )
                sq_tiles[g] = sq

            def folds_and_recip(g):
                GB, ND, NT, _ = meta[g]
                sq = sq_tiles[g]
                f1 = small.tile(
                    [128, NT, 128], f16, tag="f1", bufs=2, name=f"f1{g}"
                )
                sv = sq[:].rearrange("p n (e d) -> p n e d", e=2)
                nc.vector.tensor_tensor(
                    out=f1[:], in0=sv[:, :, 0, :], in1=sv[:, :, 1, :], op=ALU.add
                )
                f2 = small.tile(
                    [128, NT, 64], f16, tag="f2", bufs=2, name=f"f2{g}"
                )
                f1v = f1[:].rearrange("p n (e d) -> p n e d", e=2)
                nc.vector.tensor_tensor(
                    out=f2[:], in0=f1v[:, :, 0, :], in1=f1v[:, :, 1, :], op=ALU.add
                )
                f3 = small.tile(
                    [128, NT, 32], f16, tag="f3", bufs=2, name=f"f3{g}"
                )
                f2v = f2[:].rearrange("p n (e d) -> p n e d", e=2)
                nc.vector.tensor_tensor(
                    out=f3[:], in0=f2v[:, :, 0, :], in1=f2v[:, :, 1, :], op=ALU.add
                )
                ssq = small.tile([128, NT], f32, tag=f"ssq{g}", name=f"ssq{g}")
                nc.vector.tensor_reduce(
                    out=ssq[:], in_=f3[:], axis=mybir.AxisListType.X, op=ALU.add
                )
                rsum = small.tile([128, NT], f32, tag=f"rs{g}", name=f"rs{g}")
                nc.vector.reciprocal(out=rsum[:], in_=ssq[:])
                return rsum

            def sqrt_stage(g, rsum):
                GB, ND, NT, _ = meta[g]
                rnorm16 = small.tile(
                    [128, NT], f16, tag=f"rn{g}", name=f"rn{g}"
                )
                nc.scalar.activation(out=rnorm16[:], in_=rsum[:], func=AF.Sqrt)
                return rnorm16

            def pe_burst(g, rnorm16):
                GB, ND, NT, _ = meta[g]
                for _ in range(N_FILL):
                    nc.tensor.matmul(
                        warm_ps[0:1, 0:128], warm_w[:, 0:1], warm_w[:, 0:128],
                        start=True, stop=True,
                    )
                x_t = x_tiles[g]
                for bb in range(GB):
                    for h in range(2):
                        out_col = s_ps[g][:, bb * 2 + h : bb * 2 + h + 1]
                        for c in range(CHUNKS):
                            j = bb * CHUNKS + c
                            nc.tensor.matmul(
                                out_col,
                                x_t[:, bb, c, h * 128 : (h + 1) * 128],
                                rnorm16[:, j : j + 1],
                                start=(c == 0),
                                stop=(c == CHUNKS - 1),
                            )

            def copy_stage(g):
                GB, ND, NT, g_b0 = meta[g]
                nc.scalar.copy(
                    s_all[:, 2 * g_b0 : 2 * (g_b0 + GB)], s_ps[g][:, 0 : 2 * GB]
                )

            # software pipeline schedule (emission order = per-engine order)
            squares(0)
            rsums = {}
            for g in range(NG):
                if g + 1 < NG:
                    squares(g + 1)
                rsums[g] = folds_and_recip(g)
                rn = sqrt_stage(g, rsums[g])
                pe_burst(g, rn)
                if g >= 1:
                    copy_stage(g - 1)
                if g == NG - 1:
                    copy_stage(g)

            # consolidated stores on the sync ring
            c0 = 0
            store_from = 0
            for g, (GB, _) in enumerate(GROUPS):
                c1 = c0 + 2 * GB
                if g in STORE_AFTER:
                    nc.sync.dma_start(
                        out=s_d[:, store_from:c1], in_=s_all[:, store_from:c1]
                    )
                    store_from = c1
                c0 = c1
    return nc


def _make_nc():
    import concourse.bacc as bacc

    nc = bacc.Bacc(trn_type="TRN2")
    _build(nc)
    nc.finalize()
    return nc


def _finish(s_out):
    # s_out: [128, 2*BL] f32; s[b, 128h+p] = s_out[p, 2b+h]
    s = np.transpose(
        s_out.astype(np.float32).reshape(128, BL, 2), (1, 2, 0)
    ).reshape(BL, D)
    return -(s * s).sum(axis=-1) / np.float32(Q * Q)


def _run(x, trace=False):
    from concourse.bass_utils import run_bass_kernel_spmd

    x16 = np.ascontiguousarray(x.astype(np.float16))
    in_maps = [
        {"x": np.ascontiguousarray(x16[i * BL : (i + 1) * BL])}
        for i in range(N_CORES)
    ]
    nc = _make_nc()
    res = run_bass_kernel_spmd(
        nc, in_maps, core_ids=list(range(N_CORES)), trace=trace
    )
    out = np.concatenate([_finish(r["s_out"]) for r in res.results], axis=0)
    return out.astype(np.float32), res


def kernel(x: np.ndarray) -> np.ndarray:
    out, _ = _run(np.asarray(x, dtype=np.float32))
    return out
